# revision 1
# baseline (speedup 1.0000x reference)
"""Trainium2 Bass kernel for nn_FDB_65979287601425 (dual pyramid-pool attention).

Contract: kernel(**inputs) takes FULL inputs (B=16), shards batch across 8
NeuronCores, runs a Bass/Tile kernel per core, returns the FULL output.

Math notes (per batch b, branch t in {a, c}; q from x, K/V from psp(t)):
  psp(t)          : concat of g x g adaptive max pools, g in (7,5,3,1) -> [64, 84]
  key_t           = wk_t @ psp(t)            (bias cancels under centering)
  Kc_t            = key_t - mean_k(key_t)    (softmax-invariant shift; makes
                                              exp range safe: logits in [-30, 31])
  logits_t        = (Kc_t^T wq_t) @ x + Kc_t^T bq_t   (q-conv composed away)
  E_t             = exp(logits_t)            (ACT, per-partition bias)
  [ctx_t; s_t*64] = [val_t^T | ones] @ E_t   (PE broadcasts softmax sum to 64 rows)
  ctxn_t          = ctx_t * (1/s_t)
  G_t             = wfin_t @ ctxn_t
  out             = x + (G_a+ba+1)*c + (G_c+bc+1)*a
All heavy matmuls run in float32r (full-rate fp32 variant on the PE).
"""

import sys

sys.path.insert(0, "/opt/trn_rl_repo")

import numpy as np

import concourse.bass as bass
import concourse.bacc as bacc
import concourse.tile as tile
from concourse import mybir
from concourse.bass_utils import run_bass_kernel_spmd

f32 = mybir.dt.float32
f32r = mybir.dt.float32r
FT = mybir.ActivationFunctionType
OP = mybir.AluOpType
AX = mybir.AxisListType

B, C, H, W = 16, 64, 210, 210
HW = H * W                 # 44100
KS = 84                    # pyramid key slots (49+25+9+1)
NCORES = 8
NB = B // NCORES           # batches per core
CH = 490                   # main-loop chunk columns (44100 % 490 == 0)
PW = 512                   # psum-bank-aligned stride for paired regions
NPAIR = HW // (2 * CH)     # 45 chunk-pairs
RP = 14                    # pooling rows per chunk (even; 210 % 14 == 0)
NPCH = H // RP             # 15 pooling chunks
H2R = RP // 2              # 7 h2-rows per pooling chunk


def _build_nc(reps=1):
    nc = bacc.Bacc(trn_type="TRN2")

    x_d = nc.declare_dram_parameter("x", [NB, C, H, W], f32, isOutput=False)
    a_d = nc.declare_dram_parameter("a", [NB, C, H, W], f32, isOutput=False)
    c_d = nc.declare_dram_parameter("c", [NB, C, H, W], f32, isOutput=False)
    wq_d = nc.declare_dram_parameter("wq", [128, 64], f32, isOutput=False)
    wkT_d = nc.declare_dram_parameter("wkT", [128, 64], f32, isOutput=False)
    wvT_d = nc.declare_dram_parameter("wvT", [128, 64], f32, isOutput=False)
    waT_d = nc.declare_dram_parameter("waT", [64, 64], f32, isOutput=False)
    wcT_d = nc.declare_dram_parameter("wcT", [64, 64], f32, isOutput=False)
    fold_d = nc.declare_dram_parameter("fold", [128, 64], f32, isOutput=False)
    ident2_d = nc.declare_dram_parameter("ident2", [128, 64], f32, isOutput=False)
    ones_d = nc.declare_dram_parameter("ones84", [KS, 64], f32, isOutput=False)
    bq_d = nc.declare_dram_parameter("bq", [128, 1], f32, isOutput=False)
    bv_d = nc.declare_dram_parameter("bv", [128, 1], f32, isOutput=False)
    beta_d = nc.declare_dram_parameter("beta", [128, 1], f32, isOutput=False)
    out_d = nc.declare_dram_parameter("out", [NB, C, H, W], f32, isOutput=True)

    from contextlib import ExitStack

    with tile.TileContext(nc) as tc:
        with ExitStack() as ctx:
            ep = lambda **kw: ctx.enter_context(tc.tile_pool(**kw))
            cst = ep(name="cst", bufs=1)
            # pooling pools
            praw = ep(name="praw", bufs=2)
            ph2 = ep(name="ph2", bufs=2)
            pw2 = ep(name="pw2", bufs=2)
            pw6 = ep(name="pw6", bufs=2)
            pwfull = ep(name="pwfull", bufs=2)
            ppool = ep(name="ppool", bufs=2)
            # preamble pools
            ppre = ep(name="ppre", bufs=1, space="PSUM")
            spre = ep(name="spre", bufs=4)
            slhs = ep(name="slhs", bufs=4)
            # main-loop pools
            mx = ep(name="mx", bufs=3)
            mca = ep(name="mca", bufs=3)
            me = ep(name="me", bufs=2)
            mb = ep(name="mb", bufs=2)
            ms = ep(name="ms", bufs=2)
            mp2 = ep(name="mp2", bufs=2)
            mo = ep(name="mo", bufs=2)
            pl = ep(name="pl", bufs=1, space="PSUM")
            pac = ep(name="pac", bufs=1, space="PSUM")
            pg = ep(name="pg", bufs=1, space="PSUM")
            po = ep(name="po", bufs=1, space="PSUM")
            # ---- constants ----
            wq_t = cst.tile([128, 64], f32)
            nc.sync.dma_start(out=wq_t[:, :], in_=wq_d[:, :])
            wkT_t = cst.tile([128, 64], f32)
            nc.sync.dma_start(out=wkT_t[:, :], in_=wkT_d[:, :])
            wvT_t = cst.tile([128, 64], f32)
            nc.sync.dma_start(out=wvT_t[:, :], in_=wvT_d[:, :])
            identF = cst.tile([128, 64], f32)
            nc.sync.dma_start(out=identF[:, :], in_=ident2_d[:, :])
            bq_t = cst.tile([128, 1], f32)
            nc.sync.dma_start(out=bq_t[:, :], in_=bq_d[:, :])
            bv_t = cst.tile([128, 1], f32)
            nc.sync.dma_start(out=bv_t[:, :], in_=bv_d[:, :])
            betaA_t = cst.tile([64, 1], f32)
            nc.sync.dma_start(out=betaA_t[:, :], in_=beta_d[0:64, :])
            betaC_t = cst.tile([64, 1], f32)
            nc.sync.dma_start(out=betaC_t[:, :], in_=beta_d[64:128, :])
            # f32r-consumed constants
            waT_t = cst.tile([64, 64], f32)
            nc.sync.dma_start(out=waT_t[:, :].bitcast(f32r), in_=waT_d[:, :].bitcast(f32r))
            wcT_t = cst.tile([64, 64], f32)
            nc.sync.dma_start(out=wcT_t[:, :].bitcast(f32r), in_=wcT_d[:, :].bitcast(f32r))
            identR = cst.tile([64, 64], f32)
            nc.sync.dma_start(out=identR[:, :].bitcast(f32r), in_=ident2_d[0:64, :].bitcast(f32r))

            def pooling(b):
                """psp(a[b]) and psp(c[b]) -> pooled [128, 84] (a rows 0:64, c rows 64:128)."""
                w30 = pwfull.tile([128, 105, 7], f32, tag="w30")
                w42 = pwfull.tile([128, 105, 5], f32, tag="w42")
                w70 = pwfull.tile([128, 105, 3], f32, tag="w70")
                for ci in range(NPCH):
                    r0 = ci * RP
                    raw = praw.tile([128, RP, W], f32)
                    nc.sync.dma_start(out=raw[0:64, :, :], in_=a_d[b, :, r0 : r0 + RP, :])
                    nc.sync.dma_start(out=raw[64:128, :, :], in_=c_d[b, :, r0 : r0 + RP, :])
                    rawv = raw.rearrange("p (h two) w -> p h two w", two=2)
                    h2 = ph2.tile([128, H2R, W], f32)
                    nc.vector.tensor_tensor(h2[:, :, :], rawv[:, :, 0, :], rawv[:, :, 1, :], OP.max)
                    h2v = h2.rearrange("p h (w two) -> p h w two", two=2)
                    w2 = pw2.tile([128, H2R, 105], f32)
                    nc.vector.tensor_tensor(w2[:, :, :], h2v[:, :, :, 0], h2v[:, :, :, 1], OP.max)
                    w6 = pw6.tile([128, H2R, 35], f32)
                    nc.vector.reduce_max(
                        w6[:, :, :], w2.rearrange("p h (w win) -> p h w win", win=3), axis=AX.X
                    )
                    hsl = slice(ci * H2R, (ci + 1) * H2R)
                    nc.vector.reduce_max(
                        w30[:, hsl, :], w6.rearrange("p h (w win) -> p h w win", win=5), axis=AX.X
                    )
                    nc.vector.reduce_max(
                        w42[:, hsl, :], w6.rearrange("p h (w win) -> p h w win", win=7), axis=AX.X
                    )
                    nc.vector.reduce_max(
                        w70[:, hsl, :], w2.rearrange("p h (w win) -> p h w win", win=35), axis=AX.X
                    )
                pooled = ppool.tile([128, KS], f32)
                nc.vector.reduce_max(
                    pooled[:, 0:49], w30.rearrange("p (hb h2) w -> p hb w h2", hb=7), axis=AX.X
                )
                nc.vector.reduce_max(
                    pooled[:, 49:74], w42.rearrange("p (hb h2) w -> p hb w h2", hb=5), axis=AX.X
                )
                nc.vector.reduce_max(
                    pooled[:, 74:83], w70.rearrange("p (hb h2) w -> p hb w h2", hb=3), axis=AX.X
                )
                nc.vector.reduce_max(
                    pooled[:, 83:84], w70.rearrange("p a b -> p (a b)"), axis=AX.X
                )
                return pooled

            def preamble(pooled):
                """Per-batch K/V prep. Returns (MaT[2], biasT[2], lhsT[2]) for branches a, c."""
                kps = ppre.tile([128, KS], f32, tag="pre")
                nc.tensor.matmul(kps[0:64, :], wkT_t[0:64, :], pooled[0:64, :])
                nc.tensor.matmul(kps[64:128, :], wkT_t[64:128, :], pooled[64:128, :], tile_position=(64, 64))
                nmean = spre.tile([128, 1], f32, tag="nmean")
                nc.vector.tensor_reduce(nmean[:, :], kps[:, :], axis=AX.X, op=OP.add, negate=True)
                nc.scalar.mul(nmean[:, :], nmean[:, :], 1.0 / KS)
                kt = spre.tile([128, KS], f32, tag="kt")
                nc.vector.tensor_scalar_add(kt[:, :], kps[:, :], nmean[:, :])
                vps = ppre.tile([128, KS], f32, tag="pre")
                nc.tensor.matmul(vps[0:64, :], wvT_t[0:64, :], pooled[0:64, :])
                nc.tensor.matmul(vps[64:128, :], wvT_t[64:128, :], pooled[64:128, :], tile_position=(64, 64))
                va = spre.tile([128, KS], f32, tag="va")
                nc.scalar.activation(va[:, :], vps[:, :], FT.Identity, bias=bv_t[:, :])

                MaTs, biasTs, lhsTs = [], [], []
                for br in range(2):
                    pr = slice(64 * br, 64 * br + 64)
                    mps = ppre.tile([64, KS], f32, tag="pre")
                    nc.tensor.matmul(mps[:, :], wq_t[pr, :], kt[pr, :], tile_position=(64 * br, 0))
                    MaT = spre.tile([64, KS], f32, tag="mat")
                    nc.scalar.copy(MaT[:, :].bitcast(f32r), mps[:, :])
                    bps = ppre.tile([KS, 1], f32, tag="pre")
                    nc.tensor.matmul(bps[:, :], kt[pr, :], bq_t[pr, :], tile_position=(64 * br, 0))
                    biasT = spre.tile([KS, 1], f32, tag="biast")
                    nc.scalar.copy(biasT[:, :], bps[:, :])
                    tps = ppre.tile([KS, 64], f32, tag="pre")
                    nc.tensor.transpose(tps[:, :], va[pr, :], identF[pr, :], tile_position=(64 * br, 0))
                    lhsT = slhs.tile([KS, 128], f32, tag="lhst")
                    nc.sync.dma_start(
                        out=lhsT[:, 0:64].bitcast(f32r), in_=ones_d[:, :].bitcast(f32r)
                    )
                    nc.scalar.copy(lhsT[:, 64:128].bitcast(f32r), tps[:, :])
                    MaTs.append(MaT)
                    biasTs.append(biasT)
                    lhsTs.append(lhsT)
                return MaTs, biasTs, lhsTs

            def main_loop(b, MaTs, biasTs, lhsTs):
                xf = x_d[b].rearrange("c h w -> c (h w)")
                af = a_d[b].rearrange("c h w -> c (h w)")
                cf = c_d[b].rearrange("c h w -> c (h w)")
                of = out_d[b].rearrange("c h w -> c (h w)")
                for p in range(NPAIR):
                    sl2 = slice(2 * CH * p, 2 * CH * (p + 1))
                    x2 = mx.tile([64, 2 * CH], f32)
                    nc.sync.dma_start(out=x2[:, :].bitcast(f32r), in_=xf[:, sl2].bitcast(f32r))
                    ct2 = mca.tile([64, 2 * CH], f32, tag="ct2")
                    nc.sync.dma_start(out=ct2[:, :], in_=cf[:, sl2])
                    at2 = mca.tile([64, 2 * CH], f32, tag="at2")
                    nc.sync.dma_start(out=at2[:, :], in_=af[:, sl2])
                    ot = mo.tile([64, 2 * CH], f32)
                    for sub in range(2):
                        csl = slice(CH * sub, CH * (sub + 1))
                        psl = pl.tile([KS, 2 * PW], f32, tag="psl")
                        nc.tensor.matmul(
                            psl[:, 0:CH], MaTs[0][:, :].bitcast(f32r), x2[:, csl].bitcast(f32r)
                        )
                        nc.tensor.matmul(
                            psl[:, PW : PW + CH], MaTs[1][:, :].bitcast(f32r), x2[:, csl].bitcast(f32r)
                        )
                        E = me.tile([KS, 2 * PW], f32)
                        nc.scalar.activation(
                            E[:, 0:CH].bitcast(f32r), psl[:, 0:CH], FT.Exp, bias=biasTs[0][:, :]
                        )
                        nc.scalar.activation(
                            E[:, PW : PW + CH].bitcast(f32r), psl[:, PW : PW + CH], FT.Exp,
                            bias=biasTs[1][:, :],
                        )
                        pAC = pac.tile([128, 2 * PW], f32, tag="pac")
                        nc.tensor.matmul(
                            pAC[:, 0:CH], lhsTs[0][:, :].bitcast(f32r), E[:, 0:CH].bitcast(f32r)
                        )
                        nc.tensor.matmul(
                            pAC[:, PW : PW + CH], lhsTs[1][:, :].bitcast(f32r),
                            E[:, PW : PW + CH].bitcast(f32r),
                        )
                        binv = mb.tile([64, 2 * PW], f32)
                        nc.vector.reciprocal_approx_fast(binv[:, 0:CH], pAC[0:64, 0:CH])
                        nc.vector.reciprocal_approx_fast(binv[:, PW : PW + CH], pAC[0:64, PW : PW + CH])
                        sU = ms.tile([64, 2 * PW], f32)
                        nc.vector.tensor_tensor(sU.rearrange("p (two w) -> p two w", two=2)[:, :, 0:CH].bitcast(f32r), pAC[64:128, :].rearrange("p (two w) -> p two w", two=2)[:, :, 0:CH], binv.rearrange("p (two w) -> p two w", two=2)[:, :, 0:CH], OP.mult)
                        pG = pg.tile([64, 2 * PW], f32, tag="pg")
                        nc.tensor.matmul(
                            pG[:, 0:CH], waT_t[:, :].bitcast(f32r), sU[:, 0:CH].bitcast(f32r)
                        )
                        nc.tensor.matmul(
                            pG[:, PW : PW + CH], wcT_t[:, :].bitcast(f32r),
                            sU[:, PW : PW + CH].bitcast(f32r),
                        )
                        P2 = mp2.tile([64, 2 * CH], f32)
                        nc.vector.scalar_tensor_tensor(
                            out=P2[:, 0:CH].bitcast(f32r), in0=pG[:, 0:CH], scalar=betaA_t[:, :],
                            in1=ct2[:, csl], op0=OP.add, op1=OP.mult,
                        )
                        nc.vector.scalar_tensor_tensor(
                            out=P2[:, CH : 2 * CH].bitcast(f32r), in0=pG[:, PW : PW + CH],
                            scalar=betaC_t[:, :], in1=at2[:, csl], op0=OP.add, op1=OP.mult,
                        )
                        pO = po.tile([64, CH], f32, tag="po")
                        nc.tensor.matmul(
                            pO[:, :], identR[:, :].bitcast(f32r), P2[:, 0:CH].bitcast(f32r),
                            start=True, stop=False,
                        )
                        nc.tensor.matmul(
                            pO[:, :], identR[:, :].bitcast(f32r), P2[:, CH : 2 * CH].bitcast(f32r),
                            start=False, stop=False,
                        )
                        nc.tensor.matmul(
                            pO[:, :], identR[:, :].bitcast(f32r), x2[:, csl].bitcast(f32r),
                            start=False, stop=True,
                        )
                        nc.scalar.copy(ot[:, csl], pO[:, :])
                    nc.sync.dma_start(out=of[:, sl2], in_=ot[:, :])

            for _rep in range(reps):
                pooled0 = pooling(0)
                pre0 = preamble(pooled0)
                pooled1 = pooling(1)
                main_loop(0, *pre0)
                pre1 = preamble(pooled1)
                main_loop(1, *pre1)

    nc.compile()
    return nc


_NC_CACHE = None


def _get_nc():
    global _NC_CACHE
    if _NC_CACHE is None:
        _NC_CACHE = _build_nc()
    return _NC_CACHE


_NC_CACHE_R = {}


def _get_nc_reps(reps):
    if reps not in _NC_CACHE_R:
        _NC_CACHE_R[reps] = _build_nc(reps)
    return _NC_CACHE_R[reps]


def _make_consts(inputs):
    consts = {
        "wq": np.concatenate([inputs["wqa"], inputs["wqc"]], axis=0),
        "wkT": np.concatenate([inputs["wka"].T, inputs["wkc"].T], axis=0),
        "wvT": np.concatenate([inputs["wva"].T, inputs["wvc"].T], axis=0),
        "waT": np.ascontiguousarray(inputs["wa"].T),
        "wcT": np.ascontiguousarray(inputs["wc"].T),
        "fold": np.concatenate([np.eye(64, dtype=np.float32)] * 2, axis=0),
        "ident2": np.concatenate([np.eye(64, dtype=np.float32)] * 2, axis=0),
        "ones84": np.ones((KS, 64), dtype=np.float32),
        "bq": np.concatenate([inputs["bqa"], inputs["bqc"]])[:, None],
        "bv": np.concatenate([inputs["bva"], inputs["bvc"]])[:, None],
        "beta": (np.concatenate([inputs["ba"], inputs["bc"]]) + 1.0)[:, None],
    }
    return {k: np.ascontiguousarray(v, dtype=np.float32) for k, v in consts.items()}


def kernel(**inputs):
    inputs = {k: np.ascontiguousarray(np.asarray(v), dtype=np.float32) for k, v in inputs.items()}
    x, a, c = inputs["x"], inputs["a"], inputs["c"]
    consts = _make_consts(inputs)
    nc = _get_nc()
    in_maps = []
    for j in range(NCORES):
        sl = slice(NB * j, NB * (j + 1))
        m = {"x": x[sl], "a": a[sl], "c": c[sl]}
        m.update(consts)
        in_maps.append(m)
    res = run_bass_kernel_spmd(nc, in_maps, list(range(NCORES)))
    out = np.concatenate([res.results[j]["out"] for j in range(NCORES)], axis=0)
    return out



# revision 26
# speedup vs baseline: 4.3336x; 4.3336x over previous
"""Trainium2 Bass kernel for nn_FDB_65979287601425 (dual pyramid-pool attention).

Contract: kernel(**inputs) takes FULL inputs (B=16), shards batch across 8
NeuronCores, runs a Bass/Tile kernel per core, returns the FULL output.

Math notes (per batch b, branch t in {a, c}; q from x, K/V from psp(t)):
  psp(t)          : concat of g x g adaptive max pools, g in (7,5,3,1) -> [64, 84]
  key_t           = wk_t @ psp(t)            (bias cancels under centering)
  Kc_t            = key_t - mean_k(key_t)    (softmax-invariant shift; makes
                                              exp range safe: logits in [-30, 31])
  logits_t        = (Kc_t^T wq_t) @ x        (q-conv composed away)
  E_t             = exp(logits_t)            (no bias: exp(Kc_t^T bq_t) is folded
                                              multiplicatively into the ones/value
                                              lhsT rows -- softmax-exact)
  s_t             = (ones*eb_t)^T E_t        (PE broadcasts denom; branch a in
                                              partitions 0:64, branch c in 64:128)
  ctx_t           = (va_t*eb_t)^T E_t        (stacked likewise)
  sU              = ctx * (1/s)              (one [128,490] DVE op per sub)
  G               = blkdiag(wa,wc)^T @ sU    (one matmul for both branches)
  P2              = (G + beta) * [c; a]      (stacked modulation)
  out             = x + P2[0:64] + P2[64:128]  (PE partition-sum via [I;I] lhsT)

Layout: a and c are interleaved host-side into one DRAM tensor `ac` with
channels 0:64 = c, 64:128 = a (bf16) so the modulation chunk and the pooling
chunk are each ONE DMA (HWDGE is a serial ~625ns/DMA resource). Consequently
pooled block 0 = psp(c) and block 1 = psp(a); the stacked weights are ordered
to match (see _make_consts). Main-loop block 0 is branch a throughout.
Heavy matmuls in float32r; x f32; a/c stream bf16. Pooling h2/w2 maxes run on
the otherwise-idle GPSIMD (Pool) engine.
"""

import sys

sys.path.insert(0, "/opt/trn_rl_repo")

import numpy as np
import ml_dtypes

import concourse.bass as bass
import concourse.bacc as bacc
import concourse.tile as tile
from concourse import mybir
from concourse.bass_utils import run_bass_kernel_spmd

f32 = mybir.dt.float32
f32r = mybir.dt.float32r
bf16 = mybir.dt.bfloat16
FT = mybir.ActivationFunctionType
OP = mybir.AluOpType
AX = mybir.AxisListType

B, C, H, W = 16, 64, 210, 210
HW = H * W                 # 44100
KS = 84                    # pyramid key slots (49+25+9+1)
NCORES = 8
NB = B // NCORES           # batches per core
CH = 490                   # main-loop chunk columns (44100 % 490 == 0)
PW = 512                   # psum-bank-aligned stride for paired regions
GS = 5                     # subs per DMA group
GC = GS * CH               # 2450 cols per DMA group
NGRP = HW // GC            # 18 groups
RP = 14                    # pooling rows per chunk (even; 210 % 14 == 0)
NPCH = H // RP             # 15 pooling chunks
H2R = RP // 2              # 7 h2-rows per pooling chunk


def _build_nc(reps=1):
    nc = bacc.Bacc(trn_type="TRN2")

    x_d = nc.declare_dram_parameter("x", [NB, C, H, W], f32, isOutput=False)
    ac_d = nc.declare_dram_parameter("ac", [NB, 2 * C, H, W], bf16, isOutput=False)
    wq_d = nc.declare_dram_parameter("wq", [128, 64], f32, isOutput=False)
    wkT_d = nc.declare_dram_parameter("wkT", [128, 64], bf16, isOutput=False)
    wvT_d = nc.declare_dram_parameter("wvT", [128, 64], bf16, isOutput=False)
    blk_d = nc.declare_dram_parameter("blk", [128, 128], f32, isOutput=False)
    idS_d = nc.declare_dram_parameter("idS", [128, 64], f32, isOutput=False)
    ones_d = nc.declare_dram_parameter("ones84", [KS, 64], f32, isOutput=False)
    bq_d = nc.declare_dram_parameter("bq", [128, 1], f32, isOutput=False)
    bv_d = nc.declare_dram_parameter("bv", [128, 1], f32, isOutput=False)
    beta_d = nc.declare_dram_parameter("beta", [128, 1], f32, isOutput=False)
    out_d = nc.declare_dram_parameter("out", [NB, C, H, W], f32, isOutput=True)

    from contextlib import ExitStack

    with tile.TileContext(nc) as tc:
        with ExitStack() as ctx:
            ep = lambda **kw: ctx.enter_context(tc.tile_pool(**kw))
            cst = ep(name="cst", bufs=1)
            # pooling pools (bf16)
            praw = ep(name="praw", bufs=3)
            ph2 = ep(name="ph2", bufs=2)
            pw2 = ep(name="pw2", bufs=2)
            pw6 = ep(name="pw6", bufs=2)
            pwfull = ep(name="pwfull", bufs=2)
            ppool = ep(name="ppool", bufs=2)
            # preamble pools
            ppre = ep(name="ppre", bufs=1, space="PSUM")
            spre = ep(name="spre", bufs=4)
            slhs = ep(name="slhs", bufs=4)
            # main-loop SBUF pools
            mx = ep(name="mx", bufs=3)
            mca = ep(name="mca", bufs=3)
            me = ep(name="me", bufs=2)
            mb = ep(name="mb", bufs=2)
            ms = ep(name="ms", bufs=2)
            mp2 = ep(name="mp2", bufs=2)
            mo = ep(name="mo", bufs=3)
            # main-loop PSUM pools (bank budget: 2 + 2 + 1 + 1 + 1 = 7, +1 ppre)
            pl = ep(name="pl", bufs=1, space="PSUM")      # psl [84,1024]  2 banks
            pcx = ep(name="pcx", bufs=2, space="PSUM")    # pCTX [128,490] 2 banks
            psm = ep(name="psm", bufs=1, space="PSUM")    # pS   [128,490] 1 bank
            pg = ep(name="pg", bufs=1, space="PSUM")      # pG2  [128,490] 1 bank
            po = ep(name="po", bufs=1, space="PSUM")      # pO   [64,490]  1 bank
            # ---- constants ----
            wq_t = cst.tile([128, 64], f32)
            nc.sync.dma_start(out=wq_t[:, :], in_=wq_d[:, :])
            wkT_t = cst.tile([128, 64], bf16)
            nc.sync.dma_start(out=wkT_t[:, :], in_=wkT_d[:, :])
            wvT_t = cst.tile([128, 64], bf16)
            nc.sync.dma_start(out=wvT_t[:, :], in_=wvT_d[:, :])
            identF = cst.tile([128, 64], f32)
            nc.sync.dma_start(out=identF[:, :], in_=idS_d[:, :])
            ones84_t = cst.tile([KS, 64], f32)
            nc.sync.dma_start(out=ones84_t[:, :], in_=ones_d[:, :])
            bq_t = cst.tile([128, 1], f32)
            nc.sync.dma_start(out=bq_t[:, :], in_=bq_d[:, :])
            bv_t = cst.tile([128, 1], f32)
            nc.sync.dma_start(out=bv_t[:, :], in_=bv_d[:, :])
            beta_t = cst.tile([128, 1], f32)
            nc.sync.dma_start(out=beta_t[:, :], in_=beta_d[:, :])
            blk_t = cst.tile([128, 128], f32)
            nc.sync.dma_start(out=blk_t[:, :].bitcast(f32r), in_=blk_d[:, :].bitcast(f32r))
            idS_t = cst.tile([128, 64], f32)
            nc.sync.dma_start(out=idS_t[:, :].bitcast(f32r), in_=idS_d[:, :].bitcast(f32r))

            def pool_start(b):
                return {
                    "b": b,
                    "w30": pwfull.tile([128, 105, 7], bf16, tag="w30", name="w30"),
                    "w42": pwfull.tile([128, 105, 5], bf16, tag="w42", name="w42"),
                    "w70": pwfull.tile([128, 105, 3], bf16, tag="w70", name="w70"),
                }

            def pool_chunk(st, ci, mode):
                """One 14-row pooling chunk. mode="gp": gpsimd TTs + gmax chains
                (keeps DVE free while a main loop runs). mode="dve": DVE TTs +
                windowed reduce_max (for the startup batch, where DVE is idle)."""
                b = st["b"]
                w30, w42, w70 = st["w30"], st["w42"], st["w70"]
                r0 = ci * RP
                raw = praw.tile([128, RP, W], bf16)
                nc.sync.dma_start(out=raw[:, :, :], in_=ac_d[b, :, r0 : r0 + RP, :])
                rawv = raw.rearrange("p (h two) w -> p h two w", two=2)
                h2 = ph2.tile([128, H2R, W], bf16)
                nc.vector.tensor_tensor(h2[:, :, :], rawv[:, :, 0, :], rawv[:, :, 1, :], OP.max)
                h2v = h2.rearrange("p h (w two) -> p h w two", two=2)
                w2 = pw2.tile([128, H2R, 105], bf16)
                nc.vector.tensor_tensor(w2[:, :, :], h2v[:, :, :, 0], h2v[:, :, :, 1], OP.max)
                hsl = slice(ci * H2R, (ci + 1) * H2R)
                w6 = pw6.tile([128, H2R, 35], bf16)
                nc.vector.reduce_max(
                    w6[:, :, :], w2.rearrange("p h (w win) -> p h w win", win=3), axis=AX.X
                )
                nc.vector.reduce_max(
                    w30[:, hsl, :], w6.rearrange("p h (w win) -> p h w win", win=5), axis=AX.X
                )
                nc.vector.reduce_max(
                    w42[:, hsl, :], w6.rearrange("p h (w win) -> p h w win", win=7), axis=AX.X
                )
                nc.vector.reduce_max(
                    w70[:, hsl, :], w2.rearrange("p h (w win) -> p h w win", win=35), axis=AX.X
                )

            def pool_final(st):
                w30, w42, w70 = st["w30"], st["w42"], st["w70"]
                pooled = ppool.tile([128, KS], bf16)
                nc.vector.reduce_max(
                    pooled[:, 0:49], w30.rearrange("p (hb h2) w -> p hb w h2", hb=7), axis=AX.X
                )
                nc.vector.reduce_max(
                    pooled[:, 49:74], w42.rearrange("p (hb h2) w -> p hb w h2", hb=5), axis=AX.X
                )
                nc.vector.reduce_max(
                    pooled[:, 74:83], w70.rearrange("p (hb h2) w -> p hb w h2", hb=3), axis=AX.X
                )
                nc.vector.reduce_max(
                    pooled[:, 83:84], w70.rearrange("p a b -> p (a b)"), axis=AX.X
                )
                return pooled

            def preamble(pooled):
                """Per-batch K/V prep. Block 0 of pooled/kps = branch c, block 1 =
                branch a. Returned lists are ordered [branch a, branch c] to match
                the main loop's partition-block convention."""
                kps = ppre.tile([128, KS], f32, tag="pre")
                nc.tensor.matmul(kps[0:64, :], wkT_t[0:64, :], pooled[0:64, :])
                nc.tensor.matmul(kps[64:128, :], wkT_t[64:128, :], pooled[64:128, :], tile_position=(64, 64))
                nmean = spre.tile([128, 1], f32, tag="nmean")
                nc.vector.tensor_reduce(nmean[:, :], kps[:, :], axis=AX.X, op=OP.add, negate=True)
                nc.scalar.mul(nmean[:, :], nmean[:, :], 1.0 / KS)
                kt = spre.tile([128, KS], f32, tag="kt")
                nc.vector.tensor_scalar_add(kt[:, :], kps[:, :], nmean[:, :])
                vps = ppre.tile([128, KS], f32, tag="pre")
                nc.tensor.matmul(vps[0:64, :], wvT_t[0:64, :], pooled[0:64, :])
                nc.tensor.matmul(vps[64:128, :], wvT_t[64:128, :], pooled[64:128, :], tile_position=(64, 64))
                va = spre.tile([128, KS], f32, tag="va")
                nc.scalar.activation(va[:, :], vps[:, :], FT.Identity, bias=bv_t[:, :])

                MaTs, Sones, Svas = [], [], []
                for blk0 in (64, 0):  # branch a lives in pooled block 1 (rows 64:128)
                    pr = slice(blk0, blk0 + 64)
                    mps = ppre.tile([64, KS], f32, tag="pre")
                    nc.tensor.matmul(mps[:, :], wq_t[pr, :], kt[pr, :], tile_position=(blk0, 0))
                    MaT = spre.tile([64, KS], f32, tag="mat")
                    nc.scalar.copy(MaT[:, :].bitcast(f32r), mps[:, :])
                    bps = ppre.tile([KS, 1], f32, tag="pre")
                    nc.tensor.matmul(bps[:, :], kt[pr, :], bq_t[pr, :], tile_position=(blk0, 0))
                    eb = spre.tile([KS, 1], f32, tag="eb")
                    nc.scalar.activation(eb[:, :], bps[:, :], FT.Exp)
                    tps = ppre.tile([KS, 64], f32, tag="pre")
                    nc.tensor.transpose(tps[:, :], va[pr, :], identF[pr, :], tile_position=(blk0, 0))
                    Sva = slhs.tile([KS, 64], bf16, tag="sva")
                    nc.vector.tensor_scalar_mul(Sva[:, :], tps[:, :], eb[:, :])
                    Son = slhs.tile([KS, 64], bf16, tag="sones")
                    nc.vector.tensor_scalar_mul(Son[:, :], ones84_t[:, :], eb[:, :])
                    MaTs.append(MaT)
                    Sones.append(Son)
                    Svas.append(Sva)
                return MaTs, Sones, Svas

            def main_loop(b, MaTs, Sones, Svas, side=None):
                """side: optional {group_index: [closure, ...]} emitted after that
                group's subs -- used to spread the next batch's pooling/preamble
                through this batch's steady state so in-order engine queues never
                stall on far-future dependencies."""
                xf = x_d[b].rearrange("c h w -> c (h w)")
                acf = ac_d[b].rearrange("c h w -> c (h w)")
                of = out_d[b].rearrange("c h w -> c (h w)")

                def stageA(st):
                    """Logits matmuls + exp."""
                    x2, csl = st["gt"]["x2"], st["csl"]
                    psl = pl.tile([KS, 2 * PW], f32, tag="psl")
                    nc.tensor.matmul(
                        psl[:, 0:CH], MaTs[0][:, :].bitcast(f32r), x2[:, csl].bitcast(f32r)
                    )
                    nc.tensor.matmul(
                        psl[:, PW : PW + CH], MaTs[1][:, :].bitcast(f32r), x2[:, csl].bitcast(f32r)
                    )
                    E = me.tile([KS, 2 * PW], bf16)
                    nc.scalar.activation(
                        E.rearrange("p (two w) -> p two w", two=2)[:, :, 0:CH],
                        psl.rearrange("p (two w) -> p two w", two=2)[:, :, 0:CH],
                        FT.Exp,
                    )
                    st["E"] = E

                def stageB1(st):
                    """Sums/ctx matmuls + reciprocal of the softmax denominators."""
                    E = st["E"]
                    pS = psm.tile([128, CH], f32, tag="ps")
                    nc.tensor.matmul(
                        pS[0:64, :], Sones[0][:, :], E[:, 0:CH], tile_position=(0, 0)
                    )
                    nc.tensor.matmul(
                        pS[64:128, :], Sones[1][:, :], E[:, PW : PW + CH],
                        tile_position=(0, 64),
                    )
                    pCTX = pcx.tile([128, CH], f32, tag="pctx")
                    nc.tensor.matmul(
                        pCTX[0:64, :], Svas[0][:, :], E[:, 0:CH], tile_position=(0, 0)
                    )
                    nc.tensor.matmul(
                        pCTX[64:128, :], Svas[1][:, :], E[:, PW : PW + CH],
                        tile_position=(0, 64),
                    )
                    binv = mb.tile([128, CH], f32)
                    nc.vector.reciprocal_approx_fast(binv[:, :], pS[:, :])
                    st["pCTX"] = pCTX
                    st["binv"] = binv

                def stageB2(st):
                    """Normalize: sU = ctx * (1/s)."""
                    sU = ms.tile([128, CH], f32)
                    nc.vector.tensor_tensor(
                        sU[:, :].bitcast(f32r), st["pCTX"][:, :], st["binv"][:, :], OP.mult
                    )
                    st["sU"] = sU

                def stageC(st):
                    """G matmul -> modulation -> final partition-sum -> ot copy."""
                    gt, csl = st["gt"], st["csl"]
                    pG2 = pg.tile([128, CH], f32, tag="pg2")
                    nc.tensor.matmul(
                        pG2[:, :], blk_t[:, :].bitcast(f32r), st["sU"][:, :].bitcast(f32r)
                    )
                    P2 = mp2.tile([128, CH], f32)
                    nc.vector.scalar_tensor_tensor(
                        out=P2[:, :].bitcast(f32r), in0=pG2[:, :], scalar=beta_t[:, :],
                        in1=gt["acT2"][:, csl], op0=OP.add, op1=OP.mult,
                    )
                    pO = po.tile([64, CH], f32, tag="po")
                    nc.tensor.matmul(
                        pO[:, :], idS_t[:, :].bitcast(f32r), P2[:, :].bitcast(f32r),
                        start=True, stop=False,
                    )
                    nc.tensor.matmul(
                        pO[:, :], idS_t[0:64, :].bitcast(f32r), gt["x2"][:, csl].bitcast(f32r),
                        start=False, stop=True,
                    )
                    nc.scalar.copy(gt["ot"][:, csl], pO[:, :])
                    if st["sub"] == GS - 1:
                        nc.sync.dma_start(out=of[:, gt["slg"]], in_=gt["ot"][:, :])

                K = NGRP * GS
                states = {}
                gts = {}

                def ensure_group(g):
                    """Issue the group's input DMAs (called one group ahead)."""
                    if g in gts or g >= NGRP:
                        return
                    slg = slice(GC * g, GC * (g + 1))
                    x2 = mx.tile([64, GC], f32)
                    nc.sync.dma_start(out=x2[:, :].bitcast(f32r), in_=xf[:, slg].bitcast(f32r))
                    acT2 = mca.tile([128, GC], bf16, tag="acT2")
                    nc.sync.dma_start(out=acT2[:, :], in_=acf[:, slg])
                    ot = mo.tile([64, GC], f32)
                    gts[g] = {"x2": x2, "acT2": acT2, "ot": ot, "slg": slg}

                for k in range(K + 3):
                    if k < K:
                        g, sub = divmod(k, GS)
                        if sub == 0:
                            ensure_group(g)
                            ensure_group(g + 1)
                        states[k] = {
                            "gt": gts[g], "sub": sub, "csl": slice(CH * sub, CH * (sub + 1))
                        }
                        stageA(states[k])
                    if 0 <= k - 1 < K:
                        stageB1(states[k - 1])
                    if 0 <= k - 2 < K:
                        stageB2(states[k - 2])
                    if 0 <= k - 3 < K:
                        stageC(states[k - 3])
                        del states[k - 3]
                    if side and k < K and (k + 1) % GS == 0:
                        for fn in side.get(k // GS, ()):
                            fn()

            # Batch sequence across reps: the first batch's pooling runs
            # standalone (h2/w2 on DVE to shorten the serial startup); every
            # later batch's pooling + preamble is interleaved into the
            # preceding batch's main loop via side tasks.
            seq = [0, 1] * reps
            st0 = pool_start(seq[0])
            for ci in range(NPCH):
                # startup batch: alternate engines so DVE and gpsimd pipeline
                pool_chunk(st0, ci, "dve" if ci % 2 else "gp")
            pre_cur = preamble(pool_final(st0))
            for i, b in enumerate(seq):
                if i + 1 < len(seq):
                    nxt = seq[i + 1]
                    holder = {}
                    stN = pool_start(nxt)
                    # Spread the 15 pooling chunks over the first NGRP-2 groups,
                    # then the final reduce, then the preamble.
                    side = {}
                    ngp = NGRP - 2
                    for ci in range(NPCH):
                        g = min(ci * ngp // NPCH, ngp - 1)
                        side.setdefault(g, []).append(lambda ci=ci: pool_chunk(stN, ci, "gp"))
                    side.setdefault(NGRP - 2, []).append(
                        lambda: holder.__setitem__("pooled", pool_final(stN))
                    )
                    side.setdefault(NGRP - 1, []).append(
                        lambda: holder.__setitem__("pre", preamble(holder["pooled"]))
                    )
                    main_loop(b, *pre_cur, side=side)
                    pre_cur = holder["pre"]
                else:
                    main_loop(b, *pre_cur)

    nc.compile()
    return nc


_NC_CACHE_R = {}


def _get_nc_reps(reps):
    if reps not in _NC_CACHE_R:
        _NC_CACHE_R[reps] = _build_nc(reps)
    return _NC_CACHE_R[reps]


def _get_nc():
    return _get_nc_reps(1)


def _make_consts(inputs):
    # Pooled/preamble block order: block 0 = branch c, block 1 = branch a
    # (because ac channels 0:64 = c). Main-loop block order: 0 = a, 1 = c.
    waT = np.ascontiguousarray(inputs["wa"].T)
    wcT = np.ascontiguousarray(inputs["wc"].T)
    blk = np.zeros((128, 128), dtype=np.float32)
    blk[0:64, 0:64] = waT
    blk[64:128, 64:128] = wcT
    eye = np.eye(64, dtype=np.float32)
    consts = {
        "wq": np.concatenate([inputs["wqc"], inputs["wqa"]], axis=0).astype(np.float32),
        "wkT": np.concatenate([inputs["wkc"].T, inputs["wka"].T], axis=0).astype(ml_dtypes.bfloat16),
        "wvT": np.concatenate([inputs["wvc"].T, inputs["wva"].T], axis=0).astype(ml_dtypes.bfloat16),
        "blk": blk,
        "idS": np.concatenate([eye, eye], axis=0),
        "ones84": np.ones((KS, 64), dtype=np.float32),
        "bq": np.concatenate([inputs["bqc"], inputs["bqa"]])[:, None].astype(np.float32),
        "bv": np.concatenate([inputs["bvc"], inputs["bva"]])[:, None].astype(np.float32),
        "beta": (np.concatenate([inputs["ba"], inputs["bc"]]) + 1.0)[:, None].astype(np.float32),
    }
    return {k: np.ascontiguousarray(v) for k, v in consts.items()}


def _make_in_maps(inputs):
    x = np.ascontiguousarray(np.asarray(inputs["x"], dtype=np.float32))
    a16 = np.asarray(inputs["a"], dtype=np.float32).astype(ml_dtypes.bfloat16)
    c16 = np.asarray(inputs["c"], dtype=np.float32).astype(ml_dtypes.bfloat16)
    ac = np.ascontiguousarray(np.concatenate([c16, a16], axis=1))  # [B, 128, H, W]
    consts = _make_consts({k: np.asarray(v) for k, v in inputs.items()})
    in_maps = []
    for j in range(NCORES):
        sl = slice(NB * j, NB * (j + 1))
        m = {"x": x[sl], "ac": ac[sl]}
        m.update(consts)
        in_maps.append(m)
    return in_maps


def kernel(**inputs):
    in_maps = _make_in_maps(inputs)
    nc = _get_nc()
    res = run_bass_kernel_spmd(nc, in_maps, list(range(NCORES)))
    out = np.concatenate([res.results[j]["out"] for j in range(NCORES)], axis=0)
    return out
